# revision 50
# baseline (speedup 1.0000x reference)
"""Trainium2 Bass kernel for nn_GAT_9947144257800.

2-layer GAT, B=16, N=256. Data-parallel over B across 8 NeuronCores
(2 batches per core, no collectives).

Per core / batch / layer:
  sid2 = [Wa|Wa]^T @ xT + bsi          (doubled si^T; f32 + bf16 copies)
  bcols = interleave([Wb|Wb]^T @ xT + bsj)   (per-j-pair bias columns, f32)
  z2[:, g, :] = relu(sid2 + bcols[:, jp_g])  (bf16; DVE tensor_scalar for
      f32->bf16 at DVE-2x rate, ACT activation from the bf16 sid2 copy;
      Pool is banned: its tensor_scalar measures ~3.9us/op on HW)
  eT[2w:2w+2, 0:2N] += a2m[:, w]^T @ z2      (one 2-wide bf16 matmul per
      two j-pairs, 512 cols through one PSUM bank, 32 accumulations)
  att = max(exp(eT + ab2), exp(SLOPE*eT + SLOPE*ab2)) * adjT
      (exp(lrelu(x)) = max(exp(x), exp(SLOPE*x)); the flat-softmax global
      max is skipped entirely -- post-lrelu logits live in [-0.2, ~40] so
      f32 exp cannot over/underflow and softmax ratios are unchanged)
  D = sum(att); x = (attT.T @ h) * (1/D)     (PE agg, DVE-scaled evacuation)

All PSUM evacuations ride DVE (tensor_scalar/tensor_copy) instead of ACT:
HW-measured, ACT was the critical engine at 86% busy before the offload.
"""

import os
import sys

if "/opt/trn_rl_repo" not in sys.path:
    sys.path.insert(0, "/opt/trn_rl_repo")

import numpy as np

NOZ = os.environ.get("K_NOZ", "0") == "1"  # debug: skip z producers+MMs

B, N, IN_DIM, MEM, HID = 16, 256, 768, 300, 64
NCORES = 8
BLOC = B // NCORES  # batches per core
SLOPE = 0.01

# z-producer engine split pattern, cycled over j-pairs.
# D = DVE tensor_scalar (fused add+max, f32-in/bf16-out keeps the DVE 2x
# mode: 164ns), A = ACT activation(Relu, bias; bf16-in/out: 221ns).
# Pool is banned from the z stream: its tensor_scalar measures 3.9us/op
# on HW (8x the cost model). 9/7 HW-A/B-tested best vs 10/6 and 11/5 --
# DVE also carries all PSUM evacuations, so ACT takes the larger z share.
Z_PATTERN = "DADDADADDADADDAD"
ZP_BUFS = 24  # z2 ring depth (steps of producer run-ahead slack)
Z_HALF_ALT = False  # engine choice per (step, half) instead of per step
BREAKS_EARLY = False  # pull next-layer prep fillers 4-8 steps earlier

_CACHE: dict = {}


def _build_nc(reps: int = 1):
    import concourse.mybir as mybir
    from concourse import bacc, tile

    f32 = mybir.dt.float32
    f32r = mybir.dt.float32r
    bf16 = mybir.dt.bfloat16
    AL = mybir.AluOpType
    AF = mybir.ActivationFunctionType
    AX = mybir.AxisListType

    nc = bacc.Bacc()

    def dp(name, shape, is_out=False):
        return nc.declare_dram_parameter(name, list(shape), f32, isOutput=is_out)

    adj_d = dp("adj", (BLOC, N, N))
    feat_d = dp("feature", (BLOC, N, IN_DIM))
    w0_d = dp("w0", (IN_DIM, MEM))
    w1_d = dp("w1p", (384, MEM))
    wsia0_d = dp("wsia0", (IN_DIM, 128))
    wsjb0_d = dp("wsjb0", (IN_DIM, 128))
    wsia1_d = dp("wsia1", (384, 128))
    wsjb1_d = dp("wsjb1", (384, 128))
    bsi0_d = dp("bsi0", (128, 1))
    bsj0_d = dp("bsj0", (128, 1))
    bsi1_d = dp("bsi1", (128, 1))
    bsj1_d = dp("bsj1", (128, 1))
    b0r_d = dp("b0row", (1, MEM))
    b1r_d = dp("b1row", (1, MEM))
    ab2_d = dp("ab2col", (128, 1))
    ab2s_d = dp("ab2scol", (128, 1))
    a2m_d = nc.declare_dram_parameter("a2m", [128, 32, 64], bf16, isOutput=False)
    id_d = dp("ident", (128, 128))
    o1r_d = dp("ones1x128", (1, 128))
    o1c_d = dp("ones128col", (128, 1))
    out_d = dp("out", (BLOC, N, MEM), is_out=True)

    KT0 = [(0, 128), (1, 128), (2, 128), (3, 128), (4, 128), (5, 128)]
    KT1 = [(0, 128), (1, 128), (2, 44)]
    MC = [(0, 0, 128), (1, 128, 128), (2, 256, 44)]  # (mc, m0, cp) chunks of 300

    with tile.TileContext(nc) as tc:
        import contextlib

        with contextlib.ExitStack() as ctx:
            wp = ctx.enter_context(tc.tile_pool(name="wconst", bufs=1))
            iop = ctx.enter_context(tc.tile_pool(name="io", bufs=2))
            adjp = ctx.enter_context(tc.tile_pool(name="adjp", bufs=2))
            xtp = ctx.enter_context(tc.tile_pool(name="xtp", bufs=2))
            work = ctx.enter_context(tc.tile_pool(name="work", bufs=3))
            zp = ctx.enter_context(tc.tile_pool(name="zp", bufs=ZP_BUFS))
            smp = ctx.enter_context(tc.tile_pool(name="smp", bufs=2))
            ps_eT = ctx.enter_context(tc.tile_pool(name="ps_eT", bufs=4, space="PSUM"))
            ps_mm = ctx.enter_context(tc.tile_pool(name="ps_mm", bufs=3, space="PSUM"))
            ps_sm = ctx.enter_context(tc.tile_pool(name="ps_sm", bufs=1, space="PSUM"))
            if reps > 1:
                # timing variant: run the whole kernel body `reps` times on
                # device so per-iteration time can be extracted from wall
                # clock (no NTFF profiling available under this axon setup)
                ctx.enter_context(tc.For_i(0, reps, 1))

            # ---- persistent constants + feature loads.
            # DMA queue order = emission order: identity, feature(b0) and the
            # fused s-projection weights go first (they gate the first
            # z-phase); everything else fills in behind them.
            idsb = wp.tile([128, 128], f32)
            nc.sync.dma_start(idsb[:], id_d[:, :])
            fnat0 = iop.tile([128, 2, IN_DIM], f32, tag="fnat", name="fnat0")
            for it in range(2):
                for dh in range(2):
                    nc.sync.dma_start(
                        fnat0[:, it, dh * 384 : (dh + 1) * 384],
                        feat_d[0, it * 128 : (it + 1) * 128, dh * 384 : (dh + 1) * 384],
                    )
            wsia0sb = wp.tile([128, 6, 128], f32r)
            nc.sync.dma_start(wsia0sb[:], wsia0_d[:, :].rearrange("(k p) m -> p k m", p=128).bitcast(f32r))
            wsjb0sb = wp.tile([128, 6, 128], f32r)
            nc.sync.dma_start(wsjb0sb[:], wsjb0_d[:, :].rearrange("(k p) m -> p k m", p=128).bitcast(f32r))
            bsi0sb = wp.tile([128, 1], f32)
            nc.sync.dma_start(bsi0sb[:], bsi0_d[:, :])
            bsj0sb = wp.tile([128, 1], f32)
            nc.sync.dma_start(bsj0sb[:], bsj0_d[:, :])
            a2msb = wp.tile([128, 32, 64], bf16)
            nc.sync.dma_start(a2msb[:], a2m_d[:, :, :])
            w0sb = wp.tile([128, 6, MEM], f32r)
            nc.sync.dma_start(w0sb[:], w0_d[:, :].rearrange("(k p) m -> p k m", p=128).bitcast(f32r))
            fnat1 = iop.tile([128, 2, IN_DIM], f32, tag="fnat", name="fnat1")
            for it in range(2):
                nc.sync.dma_start(
                    fnat1[:, it, :], feat_d[1, it * 128 : (it + 1) * 128, :]
                )
            w1sb = wp.tile([128, 3, MEM], f32r)
            nc.sync.dma_start(w1sb[:], w1_d[:, :].rearrange("(k p) m -> p k m", p=128).bitcast(f32r))
            wsia1sb = wp.tile([128, 3, 128], f32r)
            nc.sync.dma_start(wsia1sb[:], wsia1_d[:, :].rearrange("(k p) m -> p k m", p=128).bitcast(f32r))
            wsjb1sb = wp.tile([128, 3, 128], f32r)
            nc.sync.dma_start(wsjb1sb[:], wsjb1_d[:, :].rearrange("(k p) m -> p k m", p=128).bitcast(f32r))
            bsi1sb = wp.tile([128, 1], f32)
            nc.sync.dma_start(bsi1sb[:], bsi1_d[:, :])
            bsj1sb = wp.tile([128, 1], f32)
            nc.sync.dma_start(bsj1sb[:], bsj1_d[:, :])
            b0rsb = wp.tile([1, MEM], f32r)
            nc.sync.dma_start(b0rsb[:], b0r_d[:, :].bitcast(f32r))
            b1rsb = wp.tile([1, MEM], f32r)
            nc.sync.dma_start(b1rsb[:], b1r_d[:, :].bitcast(f32r))
            o1rrsb = wp.tile([1, 128], f32r)
            nc.sync.dma_start(o1rrsb[:], o1r_d[:, :].bitcast(f32r))
            ab2sb = wp.tile([128, 1], f32)
            nc.sync.dma_start(ab2sb[:], ab2_d[:, :])
            ab2ssb = wp.tile([128, 1], f32)
            nc.sync.dma_start(ab2ssb[:], ab2s_d[:, :])
            o1rsb = wp.tile([1, 128], f32)
            nc.sync.dma_start(o1rsb[:], o1r_d[:, :])
            o1csb = wp.tile([128, 1], f32)
            nc.sync.dma_start(o1csb[:], o1c_d[:, :])
            zcol = wp.tile([128, 1], f32)
            nc.vector.memset(zcol[:], 0.0)

            xT0, adjT, negm = {}, {}, {}

            def adj_prep(b):
                anat = iop.tile([128, 2, N], f32, tag="anat")
                nc.sync.dma_start(
                    anat[:], adj_d[b, :, :].rearrange("(i p) j -> p i j", p=128)
                )
                aT = adjp.tile([128, 2, N], f32, tag="aT")
                for it in range(2):
                    for jt in range(2):
                        pt = ps_mm.tile([128, MEM], f32, tag="pt")
                        nc.tensor.transpose(
                            pt[:, 0:128],
                            anat[:, it, jt * 128 : (jt + 1) * 128],
                            idsb[:],
                        )
                        dst = aT[:, jt, it * 128 : (it + 1) * 128]
                        if jt == 0:
                            nc.scalar.copy(dst, pt[:, 0:128])
                        else:
                            nc.vector.tensor_copy(dst, pt[:, 0:128])
                adjT[b] = aT

            def _wsel(layer):
                if layer == 0:
                    return w0sb, wsia0sb, wsjb0sb, bsi0sb, bsj0sb, b0rsb
                return w1sb, wsia1sb, wsjb1sb, bsi1sb, bsj1sb, b1rsb

            def prep_s_sid(layer, xTb, ktiles):
                """si doubled, straight from xT via host-fused Wl@Wa.
                Returns (f32, bf16) copies: DVE producers read f32 (fast DVE
                mode needs f32 input), ACT producers read bf16 (ACT converts
                f32->bf16 slowly but runs bf16->bf16 at full rate)."""
                wn, wsia, wsjb, bsi, bsj, brow = _wsel(layer)
                nkt = len(ktiles)
                sid2 = work.tile([128, N], f32, tag="sid2")
                sid2b = work.tile([128, N], bf16, tag="sid2b")
                pts = ps_mm.tile([128, MEM], f32, tag="pt")
                for kt, kr in ktiles:
                    nc.tensor.matmul(
                        pts[0:128, 0:N],
                        wsia[0:kr, kt, :],
                        xTb[0:kr, kt, :],
                        start=(kt == 0),
                        stop=(kt == nkt - 1),
                    )
                nc.vector.tensor_scalar(
                    sid2[:], pts[0:128, 0:N], bsi[:, 0:1], None, AL.add
                )
                nc.vector.tensor_copy(sid2b[:], sid2[:])
                return sid2, sid2b

            def prep_s_bcols(layer, xTb, ktiles):
                """sj doubled -> interleaved per-j-pair bias columns (+ab1)."""
                wn, wsia, wsjb, bsi, bsj, brow = _wsel(layer)
                nkt = len(ktiles)
                bcols = work.tile([128, 128], f32, tag="bcols")
                ptj = ps_mm.tile([128, MEM], f32, tag="pt")
                for kt, kr in ktiles:
                    nc.tensor.matmul(
                        ptj[0:128, 0:N],
                        wsjb[0:kr, kt, :],
                        xTb[0:kr, kt, :],
                        start=(kt == 0),
                        stop=(kt == nkt - 1),
                    )
                lo = ptj[0:64, 0:N].rearrange("p (j two) -> p j two", two=2)
                hi = ptj[64:128, 0:N].rearrange("p (j two) -> p j two", two=2)
                nc.vector.tensor_scalar(
                    bcols[0:64, :], lo[:, :, 0], bsj[0:64, 0:1], None, AL.add
                )
                nc.vector.tensor_scalar(
                    bcols[64:128, :], hi[:, :, 1], bsj[64:128, 0:1], None, AL.add
                )
                return bcols

            def prep_h(layer, xTb, ktiles):
                """h natural [i, m] = x @ Wl + bl; bias applied as a rank-1
                ones x b_row matmul opening each PSUM accumulation group.
                Only needed at aggregation time, so off the critical path."""
                wn, wsia, wsjb, bsi, bsj, brow = _wsel(layer)
                nkt = len(ktiles)
                h = work.tile([128, 2, MEM], f32r, tag="h")
                for it in range(2):
                    pt = ps_mm.tile([128, MEM], f32, tag="pt")
                    nc.tensor.matmul(
                        pt[0:128, 0:MEM],
                        o1rrsb[0:1, :],
                        brow[0:1, :],
                        start=True,
                        stop=False,
                        skip_group_check=True,
                    )
                    for kt, kr in ktiles:
                        nc.tensor.matmul(
                            pt[0:128, 0:MEM],
                            xTb[0:kr, kt, it * 128 : (it + 1) * 128],
                            wn[0:kr, kt, :],
                            start=False,
                            stop=(kt == nkt - 1),
                            skip_group_check=True,
                        )
                    nc.vector.tensor_copy(h[:, it, :], pt[0:128, 0:MEM])
                return h

            Z_BREAKS = (
                (0, 6, 12, 18, 24, 32, 36, 44, 52)
                if BREAKS_EARLY
                else (0, 6, 12, 18, 24, 32, 40, 48, 56)
            )

            def z_phase(sid_pair, bcols, fillers=(), eTs=None):
                """64 steps of (2 producers -> one 2-wide bf16 matmul).
                Step (jt, w) produces z for pairs jt*64+w and jt*64+32+w into
                the two 256-col halves of one bf16 [128, 2, 256] tile, then a
                single matmul with a2m[:, w, :] streams all 512 cols into
                rows 2w:2w+2 of both column-blocks of eTs[jt] (accumulating
                over the 32 w's; a2m is zero except this w's two columns).
                `fillers` are closures emitted at fixed step breakpoints so
                their engine work interleaves with the producer stream."""
                sid2, sid2b = sid_pair
                if eTs is None:
                    eTs = [
                        ps_eT.tile([64, 2, N], f32, tag="eT", name=f"eT{i}")
                        for i in range(2)
                    ]
                np_ = len(Z_PATTERN)
                fills = list(fillers)
                res = []
                for step in range(64):
                    jt, w = divmod(step, 32)
                    for k, bp in enumerate(Z_BREAKS):
                        if step == bp and k < len(fills):
                            res.append(fills[k]())
                    if NOZ:
                        if step in (0, 32):
                            zz = zp.tile([128, 2, N], bf16, tag="z2")
                            nc.vector.memset(zz[:], 0.0)
                            nc.tensor.matmul(
                                eTs[step // 32][0:64, :, :].rearrange("p g i -> p (g i)"),
                                a2msb[:, 0, :],
                                zz[:, :, :].rearrange("p g i -> p (g i)"),
                                start=True,
                                stop=True,
                            )
                        continue
                    z2 = zp.tile([128, 2, N], bf16, tag="z2")
                    for g in range(2):
                        jp = jt * 64 + g * 32 + w
                        bc = bcols[:, jp : jp + 1]
                        ei = (step * 2 + g) if Z_HALF_ALT else jp
                        if Z_PATTERN[ei % np_] == "A":
                            nc.scalar.activation(z2[:, g, :], sid2b[:], AF.Relu, bias=bc)
                        else:
                            nc.vector.tensor_scalar(
                                z2[:, g, :], sid2[:], bc, 0.0, AL.add, AL.max
                            )
                    nc.tensor.matmul(
                        eTs[jt][0:64, :, :].rearrange("p g i -> p (g i)"),
                        a2msb[:, w, :],
                        z2[:, :, :].rearrange("p g i -> p (g i)"),
                        start=(w == 0),
                        stop=(w == 31),
                    )
                return eTs, res

            def sm_exp_tile(attA, attB, eT, jt):
                # att-unmasked = exp(lrelu(eT + ab2)) computed WITHOUT the
                # flat-softmax max subtraction: after lrelu, logits live in
                # [-0.2, ~40] so f32 exp can't over/underflow and softmax
                # ratios are unchanged. lrelu fuses into exp via
                # exp(lrelu(x)) = max(exp(x), exp(SLOPE*x)).
                for g in range(2):
                    nc.scalar.activation(
                        attA[g * 64 : (g + 1) * 64, jt, :], eT[0:64, g, 0:N],
                        AF.Exp, bias=ab2sb[g * 64 : (g + 1) * 64, 0:1],
                    )
                    nc.scalar.activation(
                        attB[g * 64 : (g + 1) * 64, jt, :], eT[0:64, g, 0:N],
                        AF.Exp, bias=ab2ssb[g * 64 : (g + 1) * 64, 0:1],
                        scale=SLOPE,
                    )

            def sm_mask_half(att, attA, attB, aT, jt):
                # att half jt = max(expA, expB) * adjT -- split per tile so
                # each DVE op waits only on its own tile's exps (short dep
                # distance; avoids head-of-line stalls in the z stream).
                nc.vector.tensor_tensor(
                    att[:, jt, :], attA[:, jt, :], attB[:, jt, :], AL.max
                )
                nc.vector.tensor_tensor(
                    att[:, jt, :], att[:, jt, :], aT[:, jt, :], AL.mult
                )

            def sm_denom(att):
                # D = sum(att); broadcast 1/D column
                rows = smp.tile([128, 1], f32, tag="rows")
                nc.vector.tensor_reduce(rows[:, 0:1], att[:], AX.XY, AL.add)
                ptd = ps_sm.tile([128, 128], f32, tag="st")
                nc.tensor.matmul(
                    ptd[0:1, 0:1], rows[:, 0:1], o1csb[:, 0:1], start=True, stop=True
                )
                dr = smp.tile([1, 1], f32, tag="dr")
                nc.vector.reciprocal(dr[0:1, 0:1], ptd[0:1, 0:1])
                ptb2 = ps_sm.tile([128, 128], f32, tag="st")
                nc.tensor.matmul(
                    ptb2[0:128, 0:1], o1rsb[0:1, :], dr[0:1, 0:1],
                    start=True, stop=True,
                )
                dscale = smp.tile([128, 1], f32, tag="dscale")
                nc.vector.tensor_copy(dscale[:], ptb2[0:128, 0:1])
                return dscale

            def agg_l0(h, att, dscale):
                x1T = xtp.tile([128, 3, N], f32r, tag="x1T")
                for mc, m0, cp in MC:
                    pt = ps_mm.tile([128, MEM], f32, tag="pt")
                    for jt in range(2):
                        nc.tensor.matmul(
                            pt[0:cp, 0:N],
                            h[:, jt, m0 : m0 + cp],
                            att[:, jt, :],
                            start=(jt == 0),
                            stop=(jt == 1),
                        )
                    nc.vector.tensor_scalar(
                        x1T[0:cp, mc, :], pt[0:cp, 0:N], dscale[0:cp, 0:1],
                        None, AL.mult,
                    )
                return x1T

            def agg_l1(b, h, att, dscale):
                for it in range(2):
                    pt = ps_mm.tile([128, MEM], f32, tag="pt")
                    for jt in range(2):
                        nc.tensor.matmul(
                            pt[0:128, 0:MEM],
                            att[:, jt, it * 128 : (it + 1) * 128],
                            h[:, jt, :],
                            start=(jt == 0),
                            stop=(jt == 1),
                        )
                    osb = smp.tile([128, MEM], f32, tag="osb")
                    nc.vector.tensor_scalar(
                        osb[:], pt[0:128, 0:MEM], dscale[:, 0:1], None, AL.mult
                    )
                    nc.sync.dma_start(out_d[b, it * 128 : (it + 1) * 128, :], osb[:])

            # ---- schedule: four z-phases back to back; every other piece
            # of work (softmax, aggregation, next prep, batch-1 input prep)
            # is a small closure emitted at a breakpoint inside some phase so
            # its engine ops interleave with that phase's producer stream.
            S, Bc, H, X = {}, {}, {}, {}

            def featT(b, it):
                fnat = fnat0 if b == 0 else fnat1
                xTb = xT0[b]
                for kt in range(6):
                    pt = ps_mm.tile([128, MEM], f32, tag="pt")
                    nc.tensor.transpose(
                        pt[:, 0:128],
                        fnat[:, it, kt * 128 : (kt + 1) * 128],
                        idsb[:],
                    )
                    dst = xTb[:, kt, it * 128 : (it + 1) * 128]
                    if kt % 2 == 0:
                        nc.scalar.copy(dst, pt[:, 0:128])
                    else:
                        nc.vector.tensor_copy(dst, pt[:, 0:128])

            xT0[0] = xtp.tile([128, 6, N], f32r, tag="xT0", name="xT0_0")
            xT0[1] = xtp.tile([128, 6, N], f32r, tag="xT0", name="xT0_1")
            featT(0, 0)
            featT(0, 1)
            S[(0, 0)] = prep_s_sid(0, xT0[0], KT0)
            Bc[(0, 0)] = prep_s_bcols(0, xT0[0], KT0)

            eT00, _ = z_phase(
                S[(0, 0)],
                Bc[(0, 0)],
                fillers=(
                    lambda: featT(1, 0),
                    lambda: featT(1, 1),
                    lambda: S.__setitem__((1, 0), prep_s_sid(0, xT0[1], KT0)),
                    lambda: Bc.__setitem__((1, 0), prep_s_bcols(0, xT0[1], KT0)),
                    lambda: adj_prep(0),
                    lambda: adj_prep(1),
                    lambda: H.__setitem__((0, 0), prep_h(0, xT0[0], KT0)),
                    lambda: H.__setitem__((1, 0), prep_h(0, xT0[1], KT0)),
                ),
            )

            def mk_fillers(eTs, b, layer, nxt):
                """9 fillers with short dependency distances: exp(tile0),
                exp(tile1), mask(tile0), mask(tile1), denom, agg,
                next prep_s(sid), next prep_s(bcols), next prep_h."""
                box = {}
                attA = smp.tile([128, 2, N], f32, tag="attA", name=f"attA{b}{layer}")
                attB = smp.tile([128, 2, N], f32, tag="attB", name=f"attB{b}{layer}")
                att = smp.tile([128, 2, N], f32r, tag="att", name=f"att{b}{layer}")

                def f1():
                    sm_exp_tile(attA, attB, eTs[0], 0)

                def f2():
                    sm_exp_tile(attA, attB, eTs[1], 1)

                def f3():
                    sm_mask_half(att, attA, attB, adjT[b], 0)

                def f4():
                    sm_mask_half(att, attA, attB, adjT[b], 1)

                def f5():
                    box["ds"] = sm_denom(att)

                def f6():
                    if layer == 0:
                        X[b] = agg_l0(H[(b, 0)], att, box["ds"])
                    else:
                        agg_l1(b, H[(b, 1)], att, box["ds"])

                def f7():
                    if nxt:
                        S[(b, 1)] = prep_s_sid(1, X[b], KT1)

                def f8():
                    if nxt:
                        Bc[(b, 1)] = prep_s_bcols(1, X[b], KT1)

                def f9():
                    if nxt:
                        H[(b, 1)] = prep_h(1, X[b], KT1)

                return (f1, f2, f3, f4, f5, f6, f7, f8, f9)

            eT10, _ = z_phase(
                S[(1, 0)], Bc[(1, 0)], fillers=mk_fillers(eT00, 0, 0, True)
            )
            eT01, _ = z_phase(
                S[(0, 1)], Bc[(0, 1)], fillers=mk_fillers(eT10, 1, 0, True)
            )
            # last phase: its own eT[0] is complete after step 31 (jt=0 done),
            # so the final softmax's first-tile exp runs inside the phase
            # (step-44 slot).
            eT11 = [
                ps_eT.tile([64, 2, N], f32, tag="eT", name=f"eT11_{i}")
                for i in range(2)
            ]
            attA11 = smp.tile([128, 2, N], f32, tag="attA", name="attA11")
            attB11 = smp.tile([128, 2, N], f32, tag="attB", name="attB11")
            att11 = smp.tile([128, 2, N], f32r, tag="att", name="att11")
            f1, f2, f3, f4, f5, f6, _, _, _ = mk_fillers(eT01, 0, 1, False)
            z_phase(
                S[(1, 1)],
                Bc[(1, 1)],
                fillers=(
                    f1, f2, f3, f4, f5, f6,
                    lambda: sm_exp_tile(attA11, attB11, eT11[0], 0),
                    lambda: sm_mask_half(att11, attA11, attB11, adjT[1], 0),
                    lambda: None,
                ),
                eTs=eT11,
            )
            sm_exp_tile(attA11, attB11, eT11[1], 1)
            sm_mask_half(att11, attA11, attB11, adjT[1], 1)
            ds11 = sm_denom(att11)
            agg_l1(1, H[(1, 1)], att11, ds11)

    nc.compile()
    return nc


def _host_params(W0, b0, W1, b1, A1, ab1, A2, ab2):
    f = np.float32
    d = np.float64
    Wa, Wb = np.asarray(A1[:MEM], d), np.asarray(A1[MEM:], d)
    a2 = np.asarray(A2, d)[:, 0]
    W0 = np.asarray(W0, d)
    W1 = np.asarray(W1, d)
    b0 = np.asarray(b0, d)
    b1 = np.asarray(b1, d)
    ab1 = np.asarray(ab1, d)

    def pad_rows(x, rows):
        out = np.zeros((rows,) + x.shape[1:], f)
        out[: x.shape[0]] = x
        return out

    def dbl(x):  # [K, 64] -> [K, 128] doubled columns
        return np.concatenate([x, x], axis=1)

    def dupcol(v):  # [64] -> [128, 1]
        return np.concatenate([v, v]).astype(f)[:, None].copy()

    w1p = pad_rows(W1.astype(f), 384)
    import ml_dtypes

    ab2v = float(np.asarray(ab2, f).reshape(-1)[0])
    a2m = np.zeros((128, 32, 64), f)
    for v in range(32):
        a2m[0:64, v, 2 * v] = a2
        a2m[64:128, v, 2 * v + 1] = a2
    a2m = a2m.astype(ml_dtypes.bfloat16)
    return dict(
        w0=np.ascontiguousarray(W0, f),
        w1p=w1p,
        wsia0=np.ascontiguousarray(dbl(W0 @ Wa), f),
        wsjb0=np.ascontiguousarray(dbl(W0 @ Wb), f),
        wsia1=pad_rows(dbl(W1 @ Wa).astype(f), 384),
        wsjb1=pad_rows(dbl(W1 @ Wb).astype(f), 384),
        bsi0=dupcol(b0 @ Wa),
        bsj0=dupcol(b0 @ Wb + ab1),
        bsi1=dupcol(b1 @ Wa),
        bsj1=dupcol(b1 @ Wb + ab1),
        b0row=np.ascontiguousarray(b0.astype(f)[None, :]),
        b1row=np.ascontiguousarray(b1.astype(f)[None, :]),
        ab2col=np.full((128, 1), ab2v, f),
        ab2scol=np.full((128, 1), 0.01 * ab2v, f),
        a2m=a2m,
        ident=np.eye(128, dtype=f),
        ones1x128=np.ones((1, 128), f),
        ones128col=np.ones((128, 1), f),
    )


def get_nc(reps: int = 1):
    key = f"nc{reps}"
    if key not in _CACHE:
        _CACHE[key] = _build_nc(reps)
    return _CACHE[key]


def kernel(adj, feature, W0, b0, W1, b1, A1, ab1, A2, ab2):
    from concourse.bass_utils import run_bass_kernel_spmd

    nc = get_nc()
    params = _host_params(W0, b0, W1, b1, A1, ab1, A2, ab2)
    f = np.float32
    adj = np.ascontiguousarray(adj, f)
    feature = np.ascontiguousarray(feature, f)
    in_maps = []
    for c in range(NCORES):
        m = dict(params)
        m["adj"] = adj[c * BLOC : (c + 1) * BLOC]
        m["feature"] = feature[c * BLOC : (c + 1) * BLOC]
        in_maps.append(m)
    r = run_bass_kernel_spmd(nc, in_maps, list(range(NCORES)))
    out = np.concatenate([r.results[c]["out"] for c in range(NCORES)], axis=0)
    return out.astype(np.float32)

